# revision 15
# baseline (speedup 1.0000x reference)
"""Trainium2 Bass kernel for nn_ClauseDecoder.

Data-parallel over clauses: each of the 8 cores handles 8192 unary and
8192 binary clauses. The full embedding table is replicated to every
core's HBM; clause node rows are fetched with indirect (gather) DMA.

Per 512-clause tile (all bf16 except PSUM/bias/scores):
  indirect-DMA gather, one [128, 256] op per (subtile, slot) into its
  own tile (pipelined) -> PE-transpose to feature-major [Kx128, 512]
  -> L1/L2/C1 matmuls (bf16, weights stationary, clauses moving) with
  fused bias+ReLU on ScalarE -> C2 (H,1) matmul -> scores [1, 512]
  f32 -> DMA out. Measured ~0.6-0.8 ms/core on trn2 (repeat-loop
  wall-delta); bound by the qPoolDynamic indirect-DMA path (one
  128-descriptor op per 128 gathered rows, 320 ops/core).

The global-embedding contribution to layer 1 is folded into the layer-1
bias on the host (global @ W1[slot_g] + b1), which removes one 256-wide
slot from the gather and the L1 contraction.
"""

import ml_dtypes
import numpy as np

from contextlib import ExitStack

import concourse.bass as bass
import concourse.tile as tile
from concourse import bacc, mybir
from concourse.bass_utils import run_bass_kernel_spmd
from concourse.masks import make_identity

H = 256
N_NODES = 100000
U = 65536
B = 65536
NCORES = 8
UC = U // NCORES  # 8192 unary clauses per core
BC = B // NCORES
NT = 512  # clause tile (matmul moving dim)
P = 128
F32 = mybir.dt.float32
F32R = mybir.dt.float32r
I32 = mybir.dt.int32

BF16 = mybir.dt.bfloat16
# matmul operand dtype: "bf16" (full-rate) / "f32r" / "f32"
MM_DT = "bf16"
MM = {"bf16": BF16, "f32r": F32R, "f32": F32}[MM_DT]
EMB_DT = BF16 if MM_DT == "bf16" else F32

# repeat count for timing builds (wraps the whole body in a For_i)
REPEAT = 1


def _build_bass(bc2_val: float):
    nc = bacc.Bacc("TRN2", target_bir_lowering=False, debug=False,
                   enable_asserts=False)

    emb = nc.dram_tensor("emb", [N_NODES, H], EMB_DT, kind="ExternalInput").ap()
    uidx = nc.dram_tensor("uidx", [P, UC // P * 2], I32, kind="ExternalInput").ap()
    bidx = nc.dram_tensor("bidx", [P, BC // P * 3], I32, kind="ExternalInput").ap()
    wu1 = nc.dram_tensor("wu1", [2 * H, 2 * H], MM, kind="ExternalInput").ap()
    wt1 = nc.dram_tensor("wt1", [3 * H, 2 * H], MM, kind="ExternalInput").ap()
    w2u = nc.dram_tensor("w2u", [2 * H, H], MM, kind="ExternalInput").ap()
    w2t = nc.dram_tensor("w2t", [2 * H, H], MM, kind="ExternalInput").ap()
    wc1 = nc.dram_tensor("wc1", [H, H], MM, kind="ExternalInput").ap()
    wc2 = nc.dram_tensor("wc2", [P, 2], MM, kind="ExternalInput").ap()
    b1u = nc.dram_tensor("b1u", [P, 4], F32, kind="ExternalInput").ap()
    b1t = nc.dram_tensor("b1t", [P, 4], F32, kind="ExternalInput").ap()
    b2u = nc.dram_tensor("b2u", [P, 2], F32, kind="ExternalInput").ap()
    b2t = nc.dram_tensor("b2t", [P, 2], F32, kind="ExternalInput").ap()
    bc1 = nc.dram_tensor("bc1", [P, 2], F32, kind="ExternalInput").ap()
    out = nc.dram_tensor("out", [2, UC], F32, kind="ExternalOutput").ap()

    with TileKernel(nc) as tk:
        tk.run(emb, uidx, bidx, wu1, wt1, w2u, w2t, wc1, wc2,
               b1u, b1t, b2u, b2t, bc1, out, bc2_val)
    nc.compile()
    return nc


class TileKernel:
    def __init__(self, nc):
        self.nc = nc
        self.ctx = ExitStack()

    def __enter__(self):
        self.tc = self.ctx.enter_context(tile.TileContext(self.nc))
        return self

    def __exit__(self, *exc):
        return self.ctx.__exit__(*exc)

    def run(self, emb, uidx, bidx, wu1, wt1, w2u, w2t, wc1, wc2,
            b1u, b1t, b2u, b2t, bc1, out, bc2_val):
        nc, tc, ctx = self.nc, self.tc, self.ctx

        consts = ctx.enter_context(tc.tile_pool(name="consts", bufs=1))
        gather_u = ctx.enter_context(tc.tile_pool(name="gather_u", bufs=24))
        gather_b = ctx.enter_context(tc.tile_pool(name="gather_b", bufs=36))
        xt_u = ctx.enter_context(tc.tile_pool(name="xt_u", bufs=2))
        xt_b = ctx.enter_context(tc.tile_pool(name="xt_b", bufs=2))
        acts = ctx.enter_context(tc.tile_pool(name="acts", bufs=2))
        outs = ctx.enter_context(tc.tile_pool(name="outs", bufs=2))
        ps_tr = ctx.enter_context(tc.tile_pool(name="ps_tr", bufs=2, space="PSUM"))
        ps_l1 = ctx.enter_context(tc.tile_pool(name="ps_l1", bufs=2, space="PSUM"))
        ps_l2 = ctx.enter_context(tc.tile_pool(name="ps_l2", bufs=2, space="PSUM"))
        ps_c1 = ctx.enter_context(tc.tile_pool(name="ps_c1", bufs=1, space="PSUM"))
        ps_c2 = ctx.enter_context(tc.tile_pool(name="ps_c2", bufs=1, space="PSUM"))

        ident = consts.tile([P, P], EMB_DT)
        make_identity(nc, ident[:])

        # --- load indices ---
        uidx_sb = consts.tile([P, UC // P * 2], I32)
        bidx_sb = consts.tile([P, BC // P * 3], I32)
        nc.sync.dma_start(out=uidx_sb[:], in_=uidx)
        nc.sync.dma_start(out=bidx_sb[:], in_=bidx)

        # --- load weights, chunked [128, :] with in-features on partitions.
        def load_w(name, dram, kin, fout):
            t = consts.tile([P, kin, fout], MM, tag=name)
            for k in range(kin):
                nc.sync.dma_start(out=t[:, k, :], in_=dram[k * P:(k + 1) * P, :])
            return t

        wu1_sb = load_w("wu1", wu1, 4, 512)
        wt1_sb = load_w("wt1", wt1, 6, 512)
        w2u_sb = load_w("w2u", w2u, 4, 256)
        w2t_sb = load_w("w2t", w2t, 4, 256)
        wc1_sb = load_w("wc1", wc1, 2, 256)
        wc2_sb = consts.tile([P, 2], MM)
        nc.sync.dma_start(out=wc2_sb[:], in_=wc2)

        bias_sb = {}
        for name, dram, m in (("b1u", b1u, 4), ("b1t", b1t, 4), ("b2u", b2u, 2),
                              ("b2t", b2t, 2), ("bc1", bc1, 2)):
            t = consts.tile([P, m], F32, tag=name)
            nc.sync.dma_start(out=t[:], in_=dram)
            bias_sb[name] = t

        def do_phase(n_slots, idx_sb, w1_sb, w2_sb, b1_sb, b2_sb, n_tiles,
                     out_row, gather_pool, xt_pool, tag):
            KW = 2 * n_slots  # L1 contraction chunks (slots * 256 / 128)
            for T in range(n_tiles):
                # gather: one [128, 256] op per (subtile, slot) pair — the HW
                # indirect DMA consumes exactly one index per partition. Each
                # op gets its OWN tile so the ops pipeline instead of
                # serializing on a shared destination tile.
                xg_tiles = []
                for ts in range(4 * n_slots):
                    xgt = gather_pool.tile([P, H], EMB_DT, tag=f"xg{tag}")
                    nc.gpsimd.indirect_dma_start(
                        out=xgt[:],
                        out_offset=None,
                        in_=emb,
                        in_offset=bass.IndirectOffsetOnAxis(
                            ap=idx_sb[:, 4 * n_slots * T + ts:
                                      4 * n_slots * T + ts + 1],
                            axis=0,
                        ),
                    )
                    xg_tiles.append(xgt)
                # transpose to feature-major: xt[:, k, :] = X^T chunk k
                xt = xt_pool.tile([P, KW, NT], MM, tag=f"xt{tag}")
                for t in range(4):
                    for k in range(KW):
                        src = xg_tiles[t * n_slots + k // 2]
                        ptr = ps_tr.tile([P, P], EMB_DT, tag="tr")
                        nc.tensor.transpose(
                            out=ptr[:],
                            in_=src[:, (k % 2) * P:(k % 2 + 1) * P],
                            identity=ident[:],
                        )
                        nc.vector.tensor_copy(
                            out=xt[:, k, t * P:(t + 1) * P], in_=ptr[:])

                # L1: [KW*128 -> 512] + bias + relu
                h1 = acts.tile([P, 4, NT], MM, tag="h1")
                for m in range(4):
                    ps = ps_l1.tile([P, NT], F32, tag="l1")
                    for k in range(KW):
                        nc.tensor.matmul(
                            out=ps[:],
                            lhsT=w1_sb[:, k, m * P:(m + 1) * P],
                            rhs=xt[:, k, :],
                            start=(k == 0), stop=(k == KW - 1),
                        )
                    nc.scalar.activation(
                        out=h1[:, m, :], in_=ps[:],
                        func=mybir.ActivationFunctionType.Relu,
                        bias=b1_sb[:, m:m + 1])

                # L2: [512 -> 256] + bias + relu (relu belongs to common layer)
                h2 = acts.tile([P, 2, NT], MM, tag="h2")
                for m in range(2):
                    ps = ps_l2.tile([P, NT], F32, tag="l2")
                    for k in range(4):
                        nc.tensor.matmul(
                            out=ps[:],
                            lhsT=w2_sb[:, k, m * P:(m + 1) * P],
                            rhs=h1[:, k, :],
                            start=(k == 0), stop=(k == 3),
                        )
                    nc.scalar.activation(
                        out=h2[:, m, :], in_=ps[:],
                        func=mybir.ActivationFunctionType.Relu,
                        bias=b2_sb[:, m:m + 1])

                # C1: [256 -> 256] + bias + relu
                h3 = acts.tile([P, 2, NT], MM, tag="h3")
                for m in range(2):
                    ps = ps_c1.tile([P, NT], F32, tag="c1")
                    for k in range(2):
                        nc.tensor.matmul(
                            out=ps[:],
                            lhsT=wc1_sb[:, k, m * P:(m + 1) * P],
                            rhs=h2[:, k, :],
                            start=(k == 0), stop=(k == 1),
                        )
                    nc.scalar.activation(
                        out=h3[:, m, :], in_=ps[:],
                        func=mybir.ActivationFunctionType.Relu,
                        bias=bias_sb["bc1"][:, m:m + 1])

                # C2: [256 -> 1]
                ps4 = ps_c2.tile([1, NT], F32, tag="c2")
                for k in range(2):
                    nc.tensor.matmul(
                        out=ps4[:],
                        lhsT=wc2_sb[:, k:k + 1],
                        rhs=h3[:, k, :],
                        start=(k == 0), stop=(k == 1),
                    )
                sc = outs.tile([1, NT], F32, tag="sc")
                nc.scalar.activation(
                    out=sc[:], in_=ps4[:],
                    func=mybir.ActivationFunctionType.Copy,
                    bias=bc2_val)
                nc.sync.dma_start(
                    out=out[out_row:out_row + 1, T * NT:(T + 1) * NT], in_=sc[:])

        def body():
            do_phase(2, uidx_sb, wu1_sb, w2u_sb, bias_sb["b1u"], bias_sb["b2u"],
                     UC // NT, 0, gather_u, xt_u, "u")
            do_phase(3, bidx_sb, wt1_sb, w2t_sb, bias_sb["b1t"], bias_sb["b2t"],
                     BC // NT, 1, gather_b, xt_b, "b")

        if REPEAT > 1:
            with tc.For_i(0, REPEAT, 1):
                body()
        else:
            body()


_NC_CACHE = {}


def _get_nc(bc2_val: float):
    key = (MM_DT, REPEAT, round(float(bc2_val), 9))
    if key not in _NC_CACHE:
        _NC_CACHE[key] = _build_bass(float(bc2_val))
    return _NC_CACHE[key]


def kernel(local_embedding, global_embedding, unary_idx, binary_idx,
           Wb1, bb1, Wb2, bb2, Wt1, bt1, Wt2, bt2, Wc1, bc1, Wc2, bc2):
    np_mm = {"bf16": ml_dtypes.bfloat16, "f32r": np.float32,
             "f32": np.float32}[MM_DT]
    np_emb = ml_dtypes.bfloat16 if MM_DT == "bf16" else np.float32
    local_embedding = np.ascontiguousarray(
        np.asarray(local_embedding, np.float32).astype(np_emb))
    g = np.asarray(global_embedding, np.float32).reshape(1, H)
    unary_idx = np.asarray(unary_idx)
    binary_idx = np.asarray(binary_idx)

    # fold the global-embedding slot of layer 1 into the bias
    bb1f = (np.asarray(bb1, np.float32)
            + (g @ np.asarray(Wb1, np.float32)[2 * H:3 * H, :]).ravel())
    bt1f = (np.asarray(bt1, np.float32)
            + (g @ np.asarray(Wt1, np.float32)[3 * H:4 * H, :]).ravel())

    def bias_tile(b, m):
        return np.ascontiguousarray(
            np.asarray(b, np.float32).reshape(m, P).T)

    wc2_t = np.ascontiguousarray(
        np.asarray(Wc2, np.float32).reshape(2, P, 1)[:, :, 0].T)

    nc = _get_nc(float(np.asarray(bc2, np.float32).ravel()[0]))

    in_maps = []
    for c in range(NCORES):
        us = unary_idx[c * UC:(c + 1) * UC].astype(np.int32)
        bs = binary_idx[c * BC:(c + 1) * BC].astype(np.int32)
        # [nt*128, s] -> [128, nt*s] with [p, t, s] = shard[128 t + p, s]
        us = np.ascontiguousarray(
            us.reshape(UC // P, P, 2).transpose(1, 0, 2).reshape(P, -1))
        bs = np.ascontiguousarray(
            bs.reshape(BC // P, P, 3).transpose(1, 0, 2).reshape(P, -1))
        in_maps.append({
            "emb": local_embedding,
            "uidx": us,
            "bidx": bs,
            "wu1": np.ascontiguousarray(
                np.asarray(Wb1, np.float32)[:2 * H].astype(np_mm)),
            "wt1": np.ascontiguousarray(
                np.asarray(Wt1, np.float32)[:3 * H].astype(np_mm)),
            "w2u": np.ascontiguousarray(np.asarray(Wb2, np.float32).astype(np_mm)),
            "w2t": np.ascontiguousarray(np.asarray(Wt2, np.float32).astype(np_mm)),
            "wc1": np.ascontiguousarray(np.asarray(Wc1, np.float32).astype(np_mm)),
            "wc2": wc2_t.astype(np_mm),
            "b1u": bias_tile(bb1f, 4),
            "b1t": bias_tile(bt1f, 4),
            "b2u": bias_tile(bb2, 2),
            "b2t": bias_tile(bt2, 2),
            "bc1": bias_tile(bc1, 2),
        })

    res = run_bass_kernel_spmd(nc, in_maps, core_ids=list(range(NCORES)))
    u_parts = [res.results[c]["out"][0] for c in range(NCORES)]
    b_parts = [res.results[c]["out"][1] for c in range(NCORES)]
    x = np.concatenate(u_parts + b_parts).astype(np.float32)
    return x.reshape(1, U + B)
